# revision 1
# baseline (speedup 1.0000x reference)
"""Chamfer loss kernel for Trainium2 (8 NeuronCores, data-parallel over batch).

Problem: a, b: [16, 3, 4096] f32 point clouds (D-major). Per batch:
  d[i, j] = ||pa_i - pb_j||^2 = xx_i + yy_j - 2 a_i . b_j
  loss += sum_i min_j d + sum_j min_i d ; final loss / 16.

Sharding: batch dim 16 -> 2 batches per core on 8 cores. Each core computes
its partial scalar; host sums the 8 partials (the "all-reduce").

The d matrix is produced directly by the PE via a stacked contraction:
  d[i, j] = sum_k L[k, i] * R[k, j]
with a hi/lo bf16 split for fp32-grade precision at bf16 PE speed (1 cyc/row):
  as = -sqrt(2)*a  split into  ashi + aslo   (bf16 limbs, exact split)
  bs = +sqrt(2)*b  split into  bshi + bslo
  as . bs = -2 a.b  with all four limb cross terms kept
  xx, yy embedded as two bf16 limbs each (row/col-constant residual ~2^-18).
K = 4*3 coord rows + 2 xx limbs + 2 yy limbs + 2 ones rows = 16.

Squared norms are computed in a points-major [128, 32*3] layout (n = p*32+t)
so the per-point reduction is a cheap free-axis reduce, then scattered into
the [1, 4096] stack rows by DMA.

Both orientations (d and d^T) are streamed so each side's min is a free-axis
reduction. Hardware constraints found empirically: a DVE op may read at most
one PSUM operand, and InstTensorTensorReduce crashes the exec unit on this
stack, so the reduction uses only tensor_tensor + tensor_reduce:
ScalarE drains each [128, 2048] PSUM tile to fp16 SBUF; the DVE folds the
row (two tiles) with 2x-mode fp16 tensor_tensor mins (2048 -> 512) and
finishes with a 1x tensor_reduce. A tunable fraction of rows instead
reduces straight from PSUM on the DVE (slower per element, but offloads
the ScalarE) -- balanced on hardware via DIRECT_OF_4.
"""

from contextlib import ExitStack

import numpy as np

import concourse.bass as bass
import concourse.bacc as bacc_mod
import concourse.mybir as mybir
import concourse.tile as tile
from concourse.bass_utils import run_bass_kernel_spmd

B, D, N = 16, 3, 4096
NCORES = 8
BPC = B // NCORES  # batches per core
P = 128            # partition tile
NJ = 512           # matmul free dim (one PSUM bank of fp32)
DT_W = 2048        # psum drain-tile width (4 banks)
NIT = N // P       # 32
NT = N // P        # points-per-partition in the points-major layout (32)
K = 16             # stacked contraction rows

F32 = mybir.dt.float32
BF16 = mybir.dt.bfloat16
F16 = mybir.dt.float16
X = mybir.AxisListType.X
MIN = mybir.AluOpType.min
MUL = mybir.AluOpType.mult
SUB = mybir.AluOpType.subtract
SQRT2 = float(np.sqrt(2.0))
BIG = 1.0e30
# Of every 4 it-rows, this many take the direct-DVE PSUM path (1x ttr);
# the rest drain via ScalarE to fp16 and fold on DVE at 2x. Tuned on HW.
DIRECT_OF_4 = 1


def _norm_limbs(nc, io, pref, src_d, eng=None):
    """Load src points-major, square, reduce -> [128, NT] norm limbs."""
    pt = io.tile([P, NT * D], F32, tag=pref + "pt")
    (eng or nc.sync).dma_start(
        out=pt[:].rearrange("p (t d) -> p t d", d=D),
        in_=src_d.rearrange("d (p t) -> p t d", t=NT))
    sq = io.tile([P, NT * D], F32, tag=pref + "sq")
    nc.scalar.square(sq[:], pt[:])
    col = io.tile([P, NT], F32, tag=pref + "col")
    nc.vector.tensor_reduce(
        col[:], sq[:].rearrange("p (t d) -> p t d", d=D), axis=X,
        op=mybir.AluOpType.add)
    hi = io.tile([P, NT], BF16, tag=pref + "hi")
    nc.scalar.copy(hi[:], col[:])
    lo = io.tile([P, NT], BF16, tag=pref + "lo")
    nc.vector.tensor_sub(lo[:], col[:], hi[:])
    return hi, lo


def _scatter_row(eng, dst_row, col):
    """DMA a [128, NT] column tile into a [1, N] stack row (n = p*NT + t)."""
    eng.dma_start(
        out=dst_row.rearrange("r (p t) -> r p t", t=NT),
        in_=col[:])


def _emit(ctx: ExitStack, tc: tile.TileContext, out_d, a_d, b_d, reps=1):
    nc = tc.nc

    const = ctx.enter_context(tc.tile_pool(name="const", bufs=1))
    io = ctx.enter_context(tc.tile_pool(name="io", bufs=1))
    lab = ctx.enter_context(tc.tile_pool(name="lab", bufs=2))
    red = ctx.enter_context(tc.tile_pool(name="red", bufs=4))
    outp = ctx.enter_context(tc.tile_pool(name="outp", bufs=1))
    ps = ctx.enter_context(tc.tile_pool(name="ps", bufs=2, space="PSUM"))

    ones128 = const.tile([P, 1], F32)
    nc.vector.memset(ones128[:], 1.0)
    ones2row = const.tile([2, N], BF16)
    nc.vector.memset(ones2row[:], 1.0)

    drain = ctx.enter_context(tc.tile_pool(name="drain", bufs=4))
    mpool = ctx.enter_context(tc.tile_pool(name="mpool", bufs=2))
    total = outp.tile([P, 1], F32)
    nc.vector.memset(total[:], 0.0)
    dummy = outp.tile([P, 1], F16)
    dummy32 = outp.tile([P, 1], F32)
    infc = outp.tile([P, 1], F32)
    nc.vector.memset(infc[:], BIG)

    for bi in [i % BPC for i in range(BPC * reps)]:
        A0 = io.tile([D, N], F32, tag="A0")
        nc.sync.dma_start(out=A0[:], in_=a_d[bi])
        B0 = io.tile([D, N], F32, tag="B0")
        nc.scalar.dma_start(out=B0[:], in_=b_d[bi])

        # hi/lo bf16 limbs of -sqrt(2)*a and +sqrt(2)*b
        ashi = io.tile([D, N], BF16, tag="ashi")
        nc.scalar.mul(ashi[:], A0[:], -SQRT2)
        aslo = io.tile([D, N], BF16, tag="aslo")
        nc.vector.scalar_tensor_tensor(
            out=aslo[:], in0=A0[:], scalar=-SQRT2, in1=ashi[:],
            op0=MUL, op1=SUB)
        bshi = io.tile([D, N], BF16, tag="bshi")
        nc.scalar.mul(bshi[:], B0[:], SQRT2)
        bslo = io.tile([D, N], BF16, tag="bslo")
        nc.vector.scalar_tensor_tensor(
            out=bslo[:], in0=B0[:], scalar=SQRT2, in1=bshi[:],
            op0=MUL, op1=SUB)

        xxh, xxl = _norm_limbs(nc, io, "a", a_d[bi], nc.sync)
        yyh, yyl = _norm_limbs(nc, io, "b", b_d[bi], nc.scalar)

        # assemble the four K=16 stacks via SBUF->SBUF DMA
        # pairs: (ashi,bshi),(ashi,bslo),(aslo,bshi),(aslo,bslo),
        #        (xxh,1),(xxl,1),(1,yyh),(1,yyl)
        LA = lab.tile([K, N], BF16, tag="LA")
        RB = lab.tile([K, N], BF16, tag="RB")
        LB = lab.tile([K, N], BF16, tag="LB")
        RA = lab.tile([K, N], BF16, tag="RA")
        for (dst, coords, cols, eng) in (
            (LA, (ashi, ashi, aslo, aslo), (xxh, xxl, None, None), nc.sync),
            (RB, (bshi, bslo, bshi, bslo), (None, None, yyh, yyl), nc.scalar),
            (LB, (bshi, bshi, bslo, bslo), (yyh, yyl, None, None), nc.gpsimd),
            (RA, (ashi, aslo, ashi, aslo), (None, None, xxh, xxl), nc.gpsimd),
        ):
            for g in range(4):
                eng.dma_start(out=dst[g * D:(g + 1) * D, :],
                              in_=coords[g][:])
            ob = dst[4 * D:4 * D + 2, :] if cols[0] is None else \
                dst[4 * D + 2:4 * D + 4, :]
            eng.dma_start(out=ob, in_=ones2row[:])
            for g in range(4):
                r = 4 * D + g
                if cols[g] is not None:
                    _scatter_row(eng, dst[r:r + 1, :], cols[g])

        # stream both orientations: PE -> PSUM f32; ACT drains to fp16
        # SBUF; DVE folds the row pair at 2x then ttr-reduces the halves.
        Am = red.tile([P, 2 * NIT], F32, tag="Am")
        nc.vector.memset(Am[:], BIG)
        Bm = red.tile([P, 2 * NIT], F32, tag="Bm")
        nc.vector.memset(Bm[:], BIG)
        for (lhs, rhs, acc) in ((LA, RB, Am), (LB, RA, Bm)):
            for it in range(NIT):
                ls = slice(it * P, (it + 1) * P)
                direct = (it % 4) < DIRECT_OF_4
                th = []
                for h in range(2):
                    dt = ps.tile([P, DT_W], F32, tag="dps")
                    for q in range(DT_W // NJ):
                        j0 = h * DT_W + q * NJ
                        nc.tensor.matmul(
                            dt[:, q * NJ:(q + 1) * NJ],
                            lhsT=lhs[:, ls],
                            rhs=rhs[:, j0:j0 + NJ],
                            start=True, stop=True)
                    if direct:
                        nc.vector.tensor_reduce(
                            acc[:, 2 * it + h:2 * it + h + 1], dt[:],
                            axis=X, op=MIN)
                    else:
                        t16 = drain.tile([P, DT_W], F16, tag="t16")
                        nc.scalar.copy(t16[:], dt[:])
                        th.append(t16)
                if not direct:
                    m1 = mpool.tile([P, DT_W], F16, tag="m1")
                    nc.vector.tensor_tensor(out=m1[:], in0=th[0][:],
                                            in1=th[1][:], op=MIN)
                    m2 = mpool.tile([P, DT_W // 2], F16, tag="m2")
                    nc.vector.tensor_tensor(out=m2[:], in0=m1[:, 0:DT_W // 2],
                                            in1=m1[:, DT_W // 2:], op=MIN)
                    m3 = mpool.tile([P, DT_W // 4], F16, tag="m3")
                    nc.vector.tensor_tensor(out=m3[:], in0=m2[:, 0:DT_W // 4],
                                            in1=m2[:, DT_W // 4:], op=MIN)
                    nc.vector.tensor_reduce(
                        acc[:, 2 * it:2 * it + 1], m3[:], axis=X, op=MIN)

        for (acc, tag) in ((Am, "as"), (Bm, "bs")):
            mins = red.tile([P, NIT], F32, tag=tag + "m")
            nc.vector.tensor_reduce(
                mins[:], acc[:].rearrange("p (i h) -> p i h", h=2),
                axis=X, op=MIN)
            ssum = red.tile([P, 1], F32, tag=tag + "s")
            nc.vector.reduce_sum(ssum[:], mins[:], axis=X)
            nc.vector.tensor_add(total[:], total[:], ssum[:])

    fin = ps.tile([1, 1], F32, tag="dps")
    nc.tensor.matmul(fin[:], lhsT=ones128[:], rhs=total[:], start=True,
                     stop=True)
    outs = outp.tile([1, 1], F32)
    nc.scalar.copy(outs[:], fin[:])
    nc.sync.dma_start(out=out_d[:], in_=outs[:])


def build_nc(reps: int = 1) -> bass.Bass:
    nc = bacc_mod.Bacc("TRN2", target_bir_lowering=False, debug=False)
    a_d = nc.dram_tensor("a", [BPC, D, N], F32, kind="ExternalInput").ap()
    b_d = nc.dram_tensor("b", [BPC, D, N], F32, kind="ExternalInput").ap()
    out_d = nc.dram_tensor("out", [1, 1], F32, kind="ExternalOutput").ap()
    with tile.TileContext(nc) as tc:
        with ExitStack() as ctx:
            _emit(ctx, tc, out_d, a_d, b_d, reps=reps)
    nc.compile()
    return nc


_RUNNER_CACHE: dict = {}


def _make_runner(reps: int = 1):
    """Compile once; return a callable (a, b) -> per-core out array [8,1,1]."""
    import jax
    import concourse.mybir as mb
    from concourse.bass2jax import (_bass_exec_p, install_neuronx_cc_hook,
                                    partition_id_tensor)
    from jax.experimental.shard_map import shard_map
    from jax.sharding import Mesh, PartitionSpec

    install_neuronx_cc_hook()
    nc = build_nc(reps=reps)
    partition_name = (nc.partition_id_tensor.name
                     if nc.partition_id_tensor else None)

    in_names, out_names, out_avals, zero_outs = [], [], [], []
    for alloc in nc.m.functions[0].allocations:
        if not isinstance(alloc, mb.MemoryLocationSet):
            continue
        if not alloc.memorylocations:
            continue
        name = alloc.memorylocations[0].name
        if alloc.kind == "ExternalInput":
            if name != partition_name:
                in_names.append(name)
        elif alloc.kind == "ExternalOutput":
            out_names.append(name)
            shape = tuple(alloc.tensor_shape)
            dtype = mb.dt.np(alloc.dtype)
            out_avals.append(jax.core.ShapedArray(shape, dtype))
            zero_outs.append(np.zeros(shape, dtype))
    n_params = len(in_names)
    all_in_names = in_names + out_names
    if partition_name is not None:
        all_in_names = all_in_names + [partition_name]

    def _body(*args):
        operands = list(args)
        if partition_name is not None:
            operands.append(partition_id_tensor())
        return tuple(_bass_exec_p.bind(
            *operands,
            out_avals=tuple(out_avals),
            in_names=tuple(all_in_names),
            out_names=tuple(out_names),
            lowering_input_output_aliases=(),
            sim_require_finite=True,
            sim_require_nnan=True,
            nc=nc,
        ))

    devices = jax.devices()[:NCORES]
    mesh = Mesh(np.asarray(devices), ("core",))
    n_outs = len(out_names)
    sharded = jax.jit(
        shard_map(_body, mesh=mesh,
                  in_specs=(PartitionSpec("core"),) * (n_params + n_outs),
                  out_specs=(PartitionSpec("core"),) * n_outs,
                  check_rep=False),
        donate_argnums=tuple(range(n_params, n_params + n_outs)),
        keep_unused=True)

    def run(a, b):
        per = {"a": a, "b": b}
        concat_in = [per[nm].reshape(NCORES * BPC, D, N) for nm in in_names]
        concat_zeros = [np.zeros((NCORES * z.shape[0], *z.shape[1:]), z.dtype)
                        for z in zero_outs]
        outs = sharded(*concat_in, *concat_zeros)
        return np.asarray(outs[0])  # [8*1, 1]

    return run


def get_runner(reps: int = 1):
    if reps not in _RUNNER_CACHE:
        _RUNNER_CACHE[reps] = _make_runner(reps)
    return _RUNNER_CACHE[reps]


def kernel(a, b):
    a = np.ascontiguousarray(np.asarray(a, dtype=np.float32))
    b = np.ascontiguousarray(np.asarray(b, dtype=np.float32))
    assert a.shape == (B, D, N) and b.shape == (B, D, N)
    run = get_runner()
    outs = run(a, b)
    return np.float32(float(outs.sum()) / B)



# revision 3
# speedup vs baseline: 1.0853x; 1.0853x over previous
"""Chamfer loss kernel for Trainium2 (8 NeuronCores, data-parallel batch).

Problem: a, b: [16, 3, 4096] f32 point clouds (D-major). Per batch:
  d[i, j] = xx_i + yy_j - 2 a_i . b_j
  loss += sum_i min_j d + sum_j min_i d ; final loss / 16.
Sharding: 2 batches per core; host sums the 8 per-core partials.

Single-orientation design: d is streamed ONCE per batch as 32 PSUM tiles
[128 i x 4096 j] produced by a K=13 stacked contraction:
  rows 0-8:  3-term bf16 limb expansion of -2 a.b per coordinate
             (ahi*bhi + ahi*blo + alo*bhi; the dropped lo*lo term is
             ~2^-17 relative),
  rows 9-12: xx/yy bf16 limbs paired with ones rows.
The same two stacks would serve d^T via lhsT/rhs swap (not needed here).

Reductions per tile: ScalarE drains PSUM -> fp16 SBUF (2x [128,2048]);
DVE does the col-min accumulate (tensor_tensor min at 2x into a
[128,4096] fp16 running accumulator; plain copy on the first tile) and
the row min via ONE fused custom DVE op (out=min(t0,t1), accum_out=
row-min) registered at import into concourse.dve_ops. Col mins finish
at batch end with PE transposes (4-up into one PSUM tile) + free-axis
reduces. Engine budget per rep (2 batches): DVE ~313us (bottleneck),
ACT ~260us, PE ~115us; HW measures ~300-330us/rep vs ~550us for the
previous 2-orientation kernel.

Limbs/norms are computed in a [8, 3, 512] points-folded layout so stack
rows assemble with short few-descriptor SBUF->SBUF DMAs (the previous
kernel's 128-descriptor scatter rows are gone). GpSimd is used ONLY as
a DMA queue: its stock tensor ops (and fused ops reading two slices of
one tile) wedge the exec unit on this stack.
"""

import os
from contextlib import ExitStack

import numpy as np

import concourse.bass as bass
import concourse.bacc as bacc_mod
import concourse.mybir as mybir
import concourse.tile as tile
from concourse.masks import make_identity

B, D, N = 16, 3, 4096
NCORES = 8
BPC = B // NCORES  # batches per core
P = 128
NJ = 512           # matmul free dim (one PSUM bank of fp32)
DT_W = 2048        # psum drain-tile width (4 banks)
NIT = N // P       # 32
KST = 13           # stacked contraction rows
PF = 8             # partition fold for limb/norm layout
FW = N // PF       # 512

F32 = mybir.dt.float32
BF16 = mybir.dt.bfloat16
F16 = mybir.dt.float16
X = mybir.AxisListType.X
MIN = mybir.AluOpType.min
MUL = mybir.AluOpType.mult
SUB = mybir.AluOpType.subtract
SQRT2 = float(np.sqrt(2.0))
FBIG = 6.0e4  # fp16-safe sentinel

# knob defaults; build_nc/get_runner accept overrides (for experiments)
DEFAULT_KNOBS = dict(
    use_fused=os.environ.get("USE_FUSED", "1") == "1",
)


_FUSED_OP = None


def _fused_min_rmin():
    """Register (once) a custom DVE op:
      out = min(in0, in1); accum_out = min over free dim of out
    Replaces the 4-op fp16 fold chain (m1/m2/m3/ttr) with one 1x pass.
    """
    global _FUSED_OP
    if _FUSED_OP is not None:
        return _FUSED_OP
    from concourse import dve_ops
    from concourse.dve_ops import DveOp
    from concourse.dve_spec import Spec, Src0, Src1, C0, minn, lower
    from concourse.dve_uop import DveOpSpec

    name = "ANT_TTMIN_RMIN"
    if name in dve_ops._SUB_OPCODE_FOR_NAME:
        _FUSED_OP = next(op for op in dve_ops.OPS if op.name == name)
        return _FUSED_OP
    spec = Spec(
        body=minn(Src0, Src1),
        accum=minn,
        accum_init=C0,  # bind s0= to the seed (FBIG)
        reference=lambda in0, in1, s0, s1, imm2: np.minimum(
            in0.astype(np.float32), in1.astype(np.float32)),
    )
    op = DveOp(name, spec, subdim=False, uops_sha={})
    dve_ops.OPS.append(op)
    row = dve_ops._CUSTOM_DVE_ROW_BASE + len(dve_ops.OPS) - 1
    assert row < 0x20
    dve_ops._SUB_OPCODE_FOR_NAME[name] = row
    dve_ops.CUSTOM_DVE_SPECS[name] = spec
    for ver in ("v3", "v4"):
        tmp = DveOpSpec(name=name, opcode=row,
                        uops=lower(spec, ver=ver), rd1_en=True)
        op.uops_sha[ver] = tmp.sha(ver)
    _FUSED_OP = op
    return op


def _limbs_and_norms(nc, io, pref, src_d, sgn, eng):
    """Load src [D,N] as [PF, D, FW]; return (hi, lo, nh, nl).

    hi/lo: bf16 limbs of sgn*sqrt2*src in [PF, (d f)] layout.
    nh/nl: bf16 limbs of the squared norm in [PF, FW] layout.
    """
    an = io.tile([PF, D * FW], F32, tag=pref + "an")
    eng.dma_start(
        out=an[:].rearrange("p (d f) -> p d f", f=FW),
        in_=src_d.rearrange("d (p f) -> p d f", f=FW))
    hi = io.tile([PF, D * FW], BF16, tag=pref + "hi")
    nc.scalar.mul(hi[:], an[:], sgn * SQRT2)
    lo = io.tile([PF, D * FW], BF16, tag=pref + "lo")
    nc.vector.scalar_tensor_tensor(
        out=lo[:], in0=an[:], scalar=sgn * SQRT2, in1=hi[:],
        op0=MUL, op1=SUB)
    sq = io.tile([PF, D * FW], F32, tag=pref + "sq")
    nc.scalar.square(sq[:], an[:])
    sqv = sq[:].rearrange("p (d f) -> p d f", f=FW)
    nn = io.tile([PF, FW], F32, tag=pref + "nn")
    nc.vector.tensor_add(nn[:], sqv[:, 0], sqv[:, 1])
    nc.vector.tensor_add(nn[:], nn[:], sqv[:, 2])
    nh = io.tile([PF, FW], BF16, tag=pref + "nh")
    nc.vector.tensor_copy(nh[:], nn[:])
    nl = io.tile([PF, FW], BF16, tag=pref + "nl")
    nc.vector.tensor_sub(nl[:], nn[:], nh[:])
    return hi, lo, nh, nl


def _row1(dst_row):
    """View a [1, N] stack row as [1, PF, FW] for norm-row DMA."""
    return dst_row.rearrange("r (p f) -> r p f", f=FW)


def _emit(ctx: ExitStack, tc: tile.TileContext, out_d, a_d, b_d, reps=1,
          kn=None):
    kn = dict(DEFAULT_KNOBS, **(kn or {}))
    nc = tc.nc

    const = ctx.enter_context(tc.tile_pool(name="const", bufs=1))
    io = ctx.enter_context(tc.tile_pool(name="io", bufs=2))
    lab = ctx.enter_context(tc.tile_pool(name="lab", bufs=2))
    red = ctx.enter_context(tc.tile_pool(name="red", bufs=2))
    drain = ctx.enter_context(tc.tile_pool(name="drain", bufs=3))
    mpool = ctx.enter_context(tc.tile_pool(name="mpool", bufs=2))
    outp = ctx.enter_context(tc.tile_pool(name="outp", bufs=1))
    ps = ctx.enter_context(tc.tile_pool(name="ps", bufs=2, space="PSUM"))

    ones128 = const.tile([P, 1], F32)
    nc.vector.memset(ones128[:], 1.0)
    ones2row = const.tile([2, N], BF16)
    nc.vector.memset(ones2row[:], 1.0)
    ident = const.tile([P, P], F16)
    make_identity(nc, ident)

    total = outp.tile([P, 1], F32)
    nc.vector.memset(total[:], 0.0)

    for bi in [i % BPC for i in range(BPC * reps)]:
        ahi, alo, xxh, xxl = _limbs_and_norms(
            nc, io, "a", a_d[bi], -1.0, nc.sync)
        bhi, blo, yyh, yyl = _limbs_and_norms(
            nc, io, "b", b_d[bi], +1.0, nc.scalar)

        # stacks: row r of sa pairs with row r of sb.
        #   0-2: ahi*bhi  3-5: ahi*blo  6-8: alo*bhi
        #   9: xxh*1  10: xxl*1  11: 1*yyh  12: 1*yyl
        sa = lab.tile([KST, N], BF16, tag="sa")
        sb = lab.tile([KST, N], BF16, tag="sb")
        for d in range(D):
            fs = slice(d * FW, (d + 1) * FW)
            nc.sync.dma_start(out=_row1(sa[d:d + 1, :]), in_=ahi[:, fs])
            nc.sync.dma_start(out=_row1(sa[3 + d:4 + d, :]), in_=ahi[:, fs])
            nc.sync.dma_start(out=_row1(sa[6 + d:7 + d, :]), in_=alo[:, fs])
            nc.gpsimd.dma_start(out=_row1(sb[d:d + 1, :]), in_=bhi[:, fs])
            nc.gpsimd.dma_start(out=_row1(sb[3 + d:4 + d, :]), in_=blo[:, fs])
            nc.gpsimd.dma_start(out=_row1(sb[6 + d:7 + d, :]), in_=bhi[:, fs])
        nc.sync.dma_start(out=_row1(sa[9:10, :]), in_=xxh[:])
        nc.sync.dma_start(out=_row1(sa[10:11, :]), in_=xxl[:])
        nc.sync.dma_start(out=sa[11:13, :], in_=ones2row[:])
        nc.gpsimd.dma_start(out=sb[9:11, :], in_=ones2row[:])
        nc.gpsimd.dma_start(out=_row1(sb[11:12, :]), in_=yyh[:])
        nc.gpsimd.dma_start(out=_row1(sb[12:13, :]), in_=yyl[:])

        cacc = red.tile([P, N], F16, tag="cacc")
        am = red.tile([P, NIT], F32, tag="am")

        fused = _fused_min_rmin() if kn['use_fused'] else None
        for it in range(NIT):
            ls = slice(it * P, (it + 1) * P)
            th = []
            for h in range(2):
                dt = ps.tile([P, DT_W], F32, tag="dps")
                for q in range(DT_W // NJ):
                    j0 = h * DT_W + q * NJ
                    nc.tensor.matmul(
                        dt[:, q * NJ:(q + 1) * NJ],
                        lhsT=sa[:, ls],
                        rhs=sb[:, j0:j0 + NJ],
                        start=True, stop=True)
                t16 = drain.tile([P, DT_W], F16, tag="t16" + str(h))
                nc.scalar.copy(t16[:], dt[:])
                cs = cacc[:, h * DT_W:(h + 1) * DT_W]
                if it == 0:
                    nc.vector.tensor_copy(cs, t16[:])
                else:
                    nc.vector.tensor_tensor(out=cs, in0=t16[:], in1=cs,
                                            op=MIN)
                th.append(t16)
            if fused is not None:
                m1 = mpool.tile([P, DT_W], F16, tag="m1")
                nc.vector._custom_dve(
                    fused, out=m1[:], in0=th[0][:],
                    in1=th[1][:], s0=FBIG, accum_out=am[:, it:it + 1])
            else:
                m1 = mpool.tile([P, DT_W], F16, tag="m1")
                nc.vector.tensor_tensor(out=m1[:], in0=th[0][:],
                                        in1=th[1][:], op=MIN)
                m2 = mpool.tile([P, DT_W // 2], F16, tag="m2")
                nc.vector.tensor_tensor(out=m2[:], in0=m1[:, 0:DT_W // 2],
                                        in1=m1[:, DT_W // 2:], op=MIN)
                m3 = mpool.tile([P, DT_W // 4], F16, tag="m3")
                nc.vector.tensor_tensor(out=m3[:], in0=m2[:, 0:DT_W // 4],
                                        in1=m2[:, DT_W // 4:], op=MIN)
                nc.vector.tensor_reduce(
                    am[:, it:it + 1], m3[:], axis=X, op=MIN)

        # col-min finalize: transpose cacc blocks 4-up, free-axis min.
        cm = red.tile([P, NIT], F32, tag="cm")
        for c4 in range(NIT // 4):
            tp = ps.tile([P, DT_W], F16, tag="dps")
            for u in range(4):
                c = c4 * 4 + u
                nc.tensor.transpose(tp[:, u * P:(u + 1) * P],
                                    cacc[:, c * P:(c + 1) * P], ident[:])
            nc.vector.tensor_reduce(
                cm[:, c4 * 4:(c4 + 1) * 4],
                tp[:, 0:4 * P].rearrange("p (u q) -> p u q", q=P),
                axis=X, op=MIN)

        for acc in (am, cm):
            ssum = red.tile([P, 1], F32, tag="ssum")
            nc.vector.reduce_sum(ssum[:], acc[:], axis=X)
            nc.vector.tensor_add(total[:], total[:], ssum[:])

    fin = ps.tile([1, 1], F32, tag="dps")
    nc.tensor.matmul(fin[:], lhsT=ones128[:], rhs=total[:], start=True,
                     stop=True)
    outs = outp.tile([1, 1], F32)
    nc.scalar.copy(outs[:], fin[:])
    nc.sync.dma_start(out=out_d[:], in_=outs[:])


def build_nc(reps: int = 1, kn=None) -> bass.Bass:
    nc = bacc_mod.Bacc("TRN2", target_bir_lowering=False, debug=False)
    a_d = nc.dram_tensor("a", [BPC, D, N], F32, kind="ExternalInput").ap()
    b_d = nc.dram_tensor("b", [BPC, D, N], F32, kind="ExternalInput").ap()
    out_d = nc.dram_tensor("out", [1, 1], F32, kind="ExternalOutput").ap()
    with tile.TileContext(nc) as tc:
        with ExitStack() as ctx:
            _emit(ctx, tc, out_d, a_d, b_d, reps=reps, kn=kn)
    nc.compile()
    return nc


_RUNNER_CACHE: dict = {}


def _make_runner(reps: int = 1, kn=None):
    """Compile once; return a callable (a, b) -> per-core out array [8,1,1]."""
    import jax
    import concourse.mybir as mb
    from concourse.bass2jax import (_bass_exec_p, install_neuronx_cc_hook,
                                    partition_id_tensor)
    from jax.experimental.shard_map import shard_map
    from jax.sharding import Mesh, PartitionSpec

    install_neuronx_cc_hook()
    nc = build_nc(reps=reps, kn=kn)
    partition_name = (nc.partition_id_tensor.name
                     if nc.partition_id_tensor else None)

    in_names, out_names, out_avals, zero_outs = [], [], [], []
    for alloc in nc.m.functions[0].allocations:
        if not isinstance(alloc, mb.MemoryLocationSet):
            continue
        if not alloc.memorylocations:
            continue
        name = alloc.memorylocations[0].name
        if alloc.kind == "ExternalInput":
            if name != partition_name:
                in_names.append(name)
        elif alloc.kind == "ExternalOutput":
            out_names.append(name)
            shape = tuple(alloc.tensor_shape)
            dtype = mb.dt.np(alloc.dtype)
            out_avals.append(jax.core.ShapedArray(shape, dtype))
            zero_outs.append(np.zeros(shape, dtype))
    n_params = len(in_names)
    all_in_names = in_names + out_names
    if partition_name is not None:
        all_in_names = all_in_names + [partition_name]

    def _body(*args):
        operands = list(args)
        if partition_name is not None:
            operands.append(partition_id_tensor())
        return tuple(_bass_exec_p.bind(
            *operands,
            out_avals=tuple(out_avals),
            in_names=tuple(all_in_names),
            out_names=tuple(out_names),
            lowering_input_output_aliases=(),
            sim_require_finite=True,
            sim_require_nnan=True,
            nc=nc,
        ))

    devices = jax.devices()[:NCORES]
    mesh = Mesh(np.asarray(devices), ("core",))
    n_outs = len(out_names)
    sharded = jax.jit(
        shard_map(_body, mesh=mesh,
                  in_specs=(PartitionSpec("core"),) * (n_params + n_outs),
                  out_specs=(PartitionSpec("core"),) * n_outs,
                  check_rep=False),
        donate_argnums=tuple(range(n_params, n_params + n_outs)),
        keep_unused=True)

    def run(a, b):
        per = {"a": a, "b": b}
        concat_in = [per[nm].reshape(NCORES * BPC, D, N) for nm in in_names]
        concat_zeros = [np.zeros((NCORES * z.shape[0], *z.shape[1:]), z.dtype)
                        for z in zero_outs]
        outs = sharded(*concat_in, *concat_zeros)
        return np.asarray(outs[0])  # [8*1, 1]

    return run


def get_runner(reps: int = 1, kn=None):
    key = (reps, tuple(sorted((kn or {}).items())))
    if key not in _RUNNER_CACHE:
        _RUNNER_CACHE[key] = _make_runner(reps, kn=kn)
    return _RUNNER_CACHE[key]


def kernel(a, b):
    a = np.ascontiguousarray(np.asarray(a, dtype=np.float32))
    b = np.ascontiguousarray(np.asarray(b, dtype=np.float32))
    assert a.shape == (B, D, N) and b.shape == (B, D, N)
    run = get_runner()
    outs = run(a, b)
    return np.float32(float(outs.sum()) / B)


# revision 4
# speedup vs baseline: 1.0974x; 1.0111x over previous
"""Chamfer loss kernel for Trainium2 (8 NeuronCores, data-parallel batch).

Problem: a, b: [16, 3, 4096] f32 point clouds (D-major). Per batch:
  d[i, j] = xx_i + yy_j - 2 a_i . b_j
  loss += sum_i min_j d + sum_j min_i d ; final loss / 16.
Sharding: 2 batches per core; host sums the 8 per-core partials.

Single-orientation design: d is streamed ONCE per batch as 32 PSUM tiles
[128 i x 4096 j] produced by a K=13 stacked contraction:
  rows 0-8:  3-term bf16 limb expansion of -2 a.b per coordinate
             (ahi*bhi + ahi*blo + alo*bhi; the dropped lo*lo term is
             ~2^-17 relative),
  rows 9-12: xx/yy bf16 limbs paired with ones rows.
The same two stacks would serve d^T via lhsT/rhs swap (not needed here).

Reductions per tile: ScalarE drains PSUM -> fp16 SBUF (2x [128,2048]);
DVE does the col-min accumulate (tensor_tensor min at 2x into a
[128,4096] fp16 running accumulator; plain copy on the first tile) and
the row min via ONE fused custom DVE op (out=min(t0,t1), accum_out=
row-min) registered at import into concourse.dve_ops. Col mins finish
at batch end with PE transposes (4-up into one PSUM tile) + free-axis
reduces. Engine budget per rep (2 batches): DVE ~313us (bottleneck),
ACT ~260us, PE ~115us; HW measures ~300-330us/rep vs ~550us for the
previous 2-orientation kernel.

Limbs/norms are computed in a [8, 3, 512] points-folded layout so stack
rows assemble with short few-descriptor SBUF->SBUF DMAs (the previous
kernel's 128-descriptor scatter rows are gone). GpSimd is used ONLY as
a DMA queue: its stock tensor ops (and fused ops reading two slices of
one tile) wedge the exec unit on this stack.
"""

import os
from contextlib import ExitStack

import numpy as np

import concourse.bass as bass
import concourse.bacc as bacc_mod
import concourse.mybir as mybir
import concourse.tile as tile
from concourse.masks import make_identity

B, D, N = 16, 3, 4096
NCORES = 8
BPC = B // NCORES  # batches per core
P = 128
NJ = 512           # matmul free dim (one PSUM bank of fp32)
DT_W = 2048        # psum drain-tile width (4 banks)
NIT = N // P       # 32
KST = 13           # stacked contraction rows
PF = 8             # partition fold for limb/norm layout
FW = N // PF       # 512

F32 = mybir.dt.float32
BF16 = mybir.dt.bfloat16
F16 = mybir.dt.float16
X = mybir.AxisListType.X
MIN = mybir.AluOpType.min
MUL = mybir.AluOpType.mult
SUB = mybir.AluOpType.subtract
SQRT2 = float(np.sqrt(2.0))
FBIG = 6.0e4  # fp16-safe sentinel

# knob defaults; build_nc/get_runner accept overrides (for experiments)
DEFAULT_KNOBS = dict(
    use_fused=os.environ.get("USE_FUSED", "1") == "1",
)


_FUSED_OP = None


def _fused_min_rmin():
    """Register (once) a custom DVE op:
      out = min(in0, in1); accum_out = min over free dim of out
    Replaces the 4-op fp16 fold chain (m1/m2/m3/ttr) with one 1x pass.
    """
    global _FUSED_OP
    if _FUSED_OP is not None:
        return _FUSED_OP
    from concourse import dve_ops
    from concourse.dve_ops import DveOp
    from concourse.dve_spec import Spec, Src0, Src1, C0, minn, lower
    from concourse.dve_uop import DveOpSpec

    name = "ANT_TTMIN_RMIN"
    if name in dve_ops._SUB_OPCODE_FOR_NAME:
        _FUSED_OP = next(op for op in dve_ops.OPS if op.name == name)
        return _FUSED_OP
    spec = Spec(
        body=minn(Src0, Src1),
        accum=minn,
        accum_init=C0,  # bind s0= to the seed (FBIG)
        reference=lambda in0, in1, s0, s1, imm2: np.minimum(
            in0.astype(np.float32), in1.astype(np.float32)),
    )
    op = DveOp(name, spec, subdim=False, uops_sha={})
    dve_ops.OPS.append(op)
    row = dve_ops._CUSTOM_DVE_ROW_BASE + len(dve_ops.OPS) - 1
    assert row < 0x20
    dve_ops._SUB_OPCODE_FOR_NAME[name] = row
    dve_ops.CUSTOM_DVE_SPECS[name] = spec
    for ver in ("v3", "v4"):
        tmp = DveOpSpec(name=name, opcode=row,
                        uops=lower(spec, ver=ver), rd1_en=True)
        op.uops_sha[ver] = tmp.sha(ver)
    _FUSED_OP = op
    return op


def _limbs_and_norms(nc, io, pref, src_d, sgn, eng):
    """Load src [D,N] as [PF, D, FW]; return (hi, lo, nh, nl).

    hi/lo: bf16 limbs of sgn*sqrt2*src in [PF, (d f)] layout.
    nh/nl: bf16 limbs of the squared norm in [PF, FW] layout.
    """
    an = io.tile([PF, D * FW], F32, tag=pref + "an")
    eng.dma_start(
        out=an[:].rearrange("p (d f) -> p d f", f=FW),
        in_=src_d.rearrange("d (p f) -> p d f", f=FW))
    hi = io.tile([PF, D * FW], BF16, tag=pref + "hi")
    nc.scalar.mul(hi[:], an[:], sgn * SQRT2)
    lo = io.tile([PF, D * FW], BF16, tag=pref + "lo")
    nc.vector.scalar_tensor_tensor(
        out=lo[:], in0=an[:], scalar=sgn * SQRT2, in1=hi[:],
        op0=MUL, op1=SUB)
    sq = io.tile([PF, D * FW], F32, tag=pref + "sq")
    nc.scalar.square(sq[:], an[:])
    sqv = sq[:].rearrange("p (d f) -> p d f", f=FW)
    nn = io.tile([PF, FW], F32, tag=pref + "nn")
    nc.vector.tensor_add(nn[:], sqv[:, 0], sqv[:, 1])
    nc.vector.tensor_add(nn[:], nn[:], sqv[:, 2])
    nh = io.tile([PF, FW], BF16, tag=pref + "nh")
    nc.vector.tensor_copy(nh[:], nn[:])
    nl = io.tile([PF, FW], BF16, tag=pref + "nl")
    nc.vector.tensor_sub(nl[:], nn[:], nh[:])
    return hi, lo, nh, nl


def _row1(dst_row):
    """View a [1, N] stack row as [1, PF, FW] for norm-row DMA."""
    return dst_row.rearrange("r (p f) -> r p f", f=FW)


def _emit(ctx: ExitStack, tc: tile.TileContext, out_d, a_d, b_d, reps=1,
          kn=None):
    kn = dict(DEFAULT_KNOBS, **(kn or {}))
    nc = tc.nc

    const = ctx.enter_context(tc.tile_pool(name="const", bufs=1))
    io = ctx.enter_context(tc.tile_pool(name="io", bufs=2))
    lab = ctx.enter_context(tc.tile_pool(name="lab", bufs=2))
    red = ctx.enter_context(tc.tile_pool(name="red", bufs=2))
    drain = ctx.enter_context(tc.tile_pool(name="drain", bufs=3))
    mpool = ctx.enter_context(tc.tile_pool(name="mpool", bufs=2))
    outp = ctx.enter_context(tc.tile_pool(name="outp", bufs=1))
    ps = ctx.enter_context(tc.tile_pool(name="ps", bufs=2, space="PSUM"))

    ones128 = const.tile([P, 1], F32)
    nc.vector.memset(ones128[:], 1.0)
    ones2row = const.tile([2, N], BF16)
    nc.vector.memset(ones2row[:], 1.0)
    ident = const.tile([P, P], F16)
    make_identity(nc, ident)

    total = outp.tile([P, 1], F32)
    nc.vector.memset(total[:], 0.0)

    for bi in [i % BPC for i in range(BPC * reps)]:
        ahi, alo, xxh, xxl = _limbs_and_norms(
            nc, io, "a", a_d[bi], -1.0, nc.sync)
        bhi, blo, yyh, yyl = _limbs_and_norms(
            nc, io, "b", b_d[bi], +1.0, nc.scalar)

        # stacks: row r of sa pairs with row r of sb.
        #   0-2: ahi*bhi  3-5: ahi*blo  6-8: alo*bhi
        #   9: xxh*1  10: xxl*1  11: 1*yyh  12: 1*yyl
        sa = lab.tile([KST, N], BF16, tag="sa")
        sb = lab.tile([KST, N], BF16, tag="sb")
        for d in range(D):
            fs = slice(d * FW, (d + 1) * FW)
            nc.sync.dma_start(out=_row1(sa[d:d + 1, :]), in_=ahi[:, fs])
            nc.sync.dma_start(out=_row1(sa[3 + d:4 + d, :]), in_=ahi[:, fs])
            nc.sync.dma_start(out=_row1(sa[6 + d:7 + d, :]), in_=alo[:, fs])
            nc.gpsimd.dma_start(out=_row1(sb[d:d + 1, :]), in_=bhi[:, fs])
            nc.gpsimd.dma_start(out=_row1(sb[3 + d:4 + d, :]), in_=blo[:, fs])
            nc.gpsimd.dma_start(out=_row1(sb[6 + d:7 + d, :]), in_=bhi[:, fs])
        nc.sync.dma_start(out=_row1(sa[9:10, :]), in_=xxh[:])
        nc.sync.dma_start(out=_row1(sa[10:11, :]), in_=xxl[:])
        nc.sync.dma_start(out=sa[11:13, :], in_=ones2row[:])
        nc.gpsimd.dma_start(out=sb[9:11, :], in_=ones2row[:])
        nc.gpsimd.dma_start(out=_row1(sb[11:12, :]), in_=yyh[:])
        nc.gpsimd.dma_start(out=_row1(sb[12:13, :]), in_=yyl[:])

        cacc = red.tile([P, N], F16, tag="cacc")
        am = red.tile([P, NIT], F32, tag="am")

        fused = _fused_min_rmin() if kn['use_fused'] else None
        for it in range(NIT):
            ls = slice(it * P, (it + 1) * P)
            th = []
            for h in range(2):
                dt = ps.tile([P, DT_W], F32, tag="dps")
                for q in range(DT_W // NJ):
                    j0 = h * DT_W + q * NJ
                    nc.tensor.matmul(
                        dt[:, q * NJ:(q + 1) * NJ],
                        lhsT=sa[:, ls],
                        rhs=sb[:, j0:j0 + NJ],
                        start=True, stop=True)
                t16 = drain.tile([P, DT_W], F16, tag="t16" + str(h))
                nc.scalar.copy(t16[:], dt[:])
                cs = cacc[:, h * DT_W:(h + 1) * DT_W]
                if it == 0:
                    # seed cacc from PSUM on ScalarE; DVE stays on row work
                    nc.scalar.copy(cs, dt[:])
                else:
                    nc.vector.tensor_tensor(out=cs, in0=t16[:], in1=cs,
                                            op=MIN)
                th.append(t16)
            if fused is not None:
                m1 = mpool.tile([P, DT_W], F16, tag="m1")
                nc.vector._custom_dve(
                    fused, out=m1[:], in0=th[0][:],
                    in1=th[1][:], s0=FBIG, accum_out=am[:, it:it + 1])
            else:
                m1 = mpool.tile([P, DT_W], F16, tag="m1")
                nc.vector.tensor_tensor(out=m1[:], in0=th[0][:],
                                        in1=th[1][:], op=MIN)
                m2 = mpool.tile([P, DT_W // 2], F16, tag="m2")
                nc.vector.tensor_tensor(out=m2[:], in0=m1[:, 0:DT_W // 2],
                                        in1=m1[:, DT_W // 2:], op=MIN)
                m3 = mpool.tile([P, DT_W // 4], F16, tag="m3")
                nc.vector.tensor_tensor(out=m3[:], in0=m2[:, 0:DT_W // 4],
                                        in1=m2[:, DT_W // 4:], op=MIN)
                nc.vector.tensor_reduce(
                    am[:, it:it + 1], m3[:], axis=X, op=MIN)

        # col-min finalize: transpose cacc blocks 16-up, free-axis min.
        cm = red.tile([P, NIT], F32, tag="cm")
        G = 16
        for cg in range(NIT // G):
            tp = ps.tile([P, G * P], F16, tag="dps")
            for u in range(G):
                c = cg * G + u
                nc.tensor.transpose(tp[:, u * P:(u + 1) * P],
                                    cacc[:, c * P:(c + 1) * P], ident[:])
            nc.vector.tensor_reduce(
                cm[:, cg * G:(cg + 1) * G],
                tp[:].rearrange("p (u q) -> p u q", q=P),
                axis=X, op=MIN)

        for acc in (am, cm):
            ssum = red.tile([P, 1], F32, tag="ssum")
            nc.vector.reduce_sum(ssum[:], acc[:], axis=X)
            nc.vector.tensor_add(total[:], total[:], ssum[:])

    fin = ps.tile([1, 1], F32, tag="dps")
    nc.tensor.matmul(fin[:], lhsT=ones128[:], rhs=total[:], start=True,
                     stop=True)
    outs = outp.tile([1, 1], F32)
    nc.scalar.copy(outs[:], fin[:])
    nc.sync.dma_start(out=out_d[:], in_=outs[:])


def build_nc(reps: int = 1, kn=None) -> bass.Bass:
    nc = bacc_mod.Bacc("TRN2", target_bir_lowering=False, debug=False)
    a_d = nc.dram_tensor("a", [BPC, D, N], F32, kind="ExternalInput").ap()
    b_d = nc.dram_tensor("b", [BPC, D, N], F32, kind="ExternalInput").ap()
    out_d = nc.dram_tensor("out", [1, 1], F32, kind="ExternalOutput").ap()
    with tile.TileContext(nc) as tc:
        with ExitStack() as ctx:
            _emit(ctx, tc, out_d, a_d, b_d, reps=reps, kn=kn)
    nc.compile()
    return nc


_RUNNER_CACHE: dict = {}


def _make_runner(reps: int = 1, kn=None):
    """Compile once; return a callable (a, b) -> per-core out array [8,1,1]."""
    import jax
    import concourse.mybir as mb
    from concourse.bass2jax import (_bass_exec_p, install_neuronx_cc_hook,
                                    partition_id_tensor)
    from jax.experimental.shard_map import shard_map
    from jax.sharding import Mesh, PartitionSpec

    install_neuronx_cc_hook()
    nc = build_nc(reps=reps, kn=kn)
    partition_name = (nc.partition_id_tensor.name
                     if nc.partition_id_tensor else None)

    in_names, out_names, out_avals, zero_outs = [], [], [], []
    for alloc in nc.m.functions[0].allocations:
        if not isinstance(alloc, mb.MemoryLocationSet):
            continue
        if not alloc.memorylocations:
            continue
        name = alloc.memorylocations[0].name
        if alloc.kind == "ExternalInput":
            if name != partition_name:
                in_names.append(name)
        elif alloc.kind == "ExternalOutput":
            out_names.append(name)
            shape = tuple(alloc.tensor_shape)
            dtype = mb.dt.np(alloc.dtype)
            out_avals.append(jax.core.ShapedArray(shape, dtype))
            zero_outs.append(np.zeros(shape, dtype))
    n_params = len(in_names)
    all_in_names = in_names + out_names
    if partition_name is not None:
        all_in_names = all_in_names + [partition_name]

    def _body(*args):
        operands = list(args)
        if partition_name is not None:
            operands.append(partition_id_tensor())
        return tuple(_bass_exec_p.bind(
            *operands,
            out_avals=tuple(out_avals),
            in_names=tuple(all_in_names),
            out_names=tuple(out_names),
            lowering_input_output_aliases=(),
            sim_require_finite=True,
            sim_require_nnan=True,
            nc=nc,
        ))

    devices = jax.devices()[:NCORES]
    mesh = Mesh(np.asarray(devices), ("core",))
    n_outs = len(out_names)
    sharded = jax.jit(
        shard_map(_body, mesh=mesh,
                  in_specs=(PartitionSpec("core"),) * (n_params + n_outs),
                  out_specs=(PartitionSpec("core"),) * n_outs,
                  check_rep=False),
        donate_argnums=tuple(range(n_params, n_params + n_outs)),
        keep_unused=True)

    def run(a, b):
        per = {"a": a, "b": b}
        concat_in = [per[nm].reshape(NCORES * BPC, D, N) for nm in in_names]
        concat_zeros = [np.zeros((NCORES * z.shape[0], *z.shape[1:]), z.dtype)
                        for z in zero_outs]
        outs = sharded(*concat_in, *concat_zeros)
        return np.asarray(outs[0])  # [8*1, 1]

    return run


def get_runner(reps: int = 1, kn=None):
    key = (reps, tuple(sorted((kn or {}).items())))
    if key not in _RUNNER_CACHE:
        _RUNNER_CACHE[key] = _make_runner(reps, kn=kn)
    return _RUNNER_CACHE[key]


def kernel(a, b):
    a = np.ascontiguousarray(np.asarray(a, dtype=np.float32))
    b = np.ascontiguousarray(np.asarray(b, dtype=np.float32))
    assert a.shape == (B, D, N) and b.shape == (B, D, N)
    run = get_runner()
    outs = run(a, b)
    return np.float32(float(outs.sum()) / B)
